# revision 2
# baseline (speedup 1.0000x reference)
"""Trainium2 Bass kernel for nn_GSA_74045236183284 (histogram_binning), v4.

Sharding: data-parallel over batch B=8 across 8 NeuronCores (1 sample/core),
params replicated, zero collectives (per-sample BatchNorm approximation).

v4 vs v3 (trace-driven):
  - Mask-row (Mrow) DRAM roundtrip + mA build + bin counts moved to the
    gpsimd engine/software-DGE rings: in v3 they shared DMAHW rings with
    the bulk x loads and blocked xb chunks until t=45us.
  - logits (both layouts) load on the fast sync HWDGE ring first; wT tanh
    no longer waits on a slow software-DGE transfer.
  - Pooled bin sums flipped: lhsT = xth slab, rhs = strided mask columns,
    PSUM out [C, 9].  Feature, r1/r2/r3 stats and sum-x then live in
    channel-major orientation: no transposes, stats are three fused
    scalar_tensor_tensor ops with accumulate.
  - gelu act-table switch pinned AFTER the last attention exp via a data
    dependency (dummy gelu reads L3's expT) - v3's emission-order hack got
    reordered by the scheduler and caused an extra mid-attention reload.
  - x^2 channel sums split scalar(10)/vector(2)/gpsimd(4) so the tail
    lands right after the last xb chunk.
  - Pass R tiles 2048 wide: fewer, cheaper gelu activations.
"""

import sys

for _p in ("/opt/trn_rl_repo",):
    if _p not in sys.path:
        sys.path.insert(0, _p)

import numpy as np

import concourse.bass as bass
import concourse.bacc as bacc
import concourse.mybir as mybir
import concourse.tile as tile
from concourse.bass_utils import run_bass_kernel_spmd

F32 = mybir.dt.float32
BF16 = mybir.dt.bfloat16
U32 = mybir.dt.uint32
AF = mybir.ActivationFunctionType
ALU = mybir.AluOpType
AX = mybir.AxisListType

B, C, N, K = 8, 128, 16384, 8
NCORES = 8
H = C // 2
XTCH = 4096     # xth DMA chunk
XBCH = 2048     # xb DMA chunk
SQCH = 2048     # x^2 op chunk
RT = 2048       # pass R tile
FT = 2048       # pass F tile


def build_nc():
    nc = bacc.Bacc("TRN2", target_bir_lowering=False, debug=False,
                   num_devices=NCORES)

    xb_d = nc.dram_tensor("xb", [C, N], BF16, kind="ExternalInput")
    xth_d = nc.dram_tensor("xth", [C, N], BF16, kind="ExternalInput")
    lg_d = nc.dram_tensor("lg", [C, C], F32, kind="ExternalInput")
    lgT_d = nc.dram_tensor("lgT", [C, C], F32, kind="ExternalInput")
    # all weight/bias params packed host-side into one [C, 14*C] tensor:
    # blocks: 0 ident, 1-9 Wq/Wk/Wv 1-3, 10 conv0_w, 11 fc1_w (rows :H),
    # 12 fc2_w (cols :H), 13 vrows (rows :7)
    params_d = nc.dram_tensor("params", [C, 14 * C], F32,
                              kind="ExternalInput")
    out_d = nc.dram_tensor("out", [C, N], BF16, kind="ExternalOutput")

    with tile.TileContext(nc) as tc:
        _body(tc, nc, xb_d, xth_d, lg_d, lgT_d, params_d, out_d)

    nc.compile()
    return nc


def _body(tc, nc, xb_d, xth_d, lg_d, lgT_d, params_d, out_d):
    from contextlib import ExitStack
    ctx = ExitStack()
    with ctx:
        singles = ctx.enter_context(tc.tile_pool(name="singles", bufs=1))
        scr2 = ctx.enter_context(tc.tile_pool(name="scr2", bufs=2))
        och = ctx.enter_context(tc.tile_pool(name="och", bufs=3))
        dramp = ctx.enter_context(tc.tile_pool(name="dramp", bufs=1,
                                               space="DRAM"))
        psA_cm = tc.tile_pool(name="psA", bufs=2, space="PSUM")
        psA = psA_cm.__enter__()

        # ------- sync HWDGE ring: logits (tiny) then xth then xb -------
        lgT = singles.tile([C, C], F32)
        nc.sync.dma_start(lgT[:], lgT_d.ap())
        lg = singles.tile([C, C], F32)
        nc.sync.dma_start(lg[:], lg_d.ap())
        # xth all on the sync queue (dispatched first -> per-ring FIFO gives
        # it bandwidth priority); xb split across sync + scalar queues, with
        # the scalar-queue half emitted after the tanh ops below.
        params = singles.tile([C, 14 * C], F32)
        nc.sync.dma_start(params[:], params_d.ap())
        xth = singles.tile([C, N], BF16)
        for ci in range(N // XTCH):
            nc.sync.dma_start(xth[:, ci * XTCH:(ci + 1) * XTCH],
                              xth_d.ap()[:, ci * XTCH:(ci + 1) * XTCH])
        # xb gated behind xth completion (WAW dep via a strided scalar copy
        # touching one column of every chunk) so xth keeps full bandwidth.
        xb = singles.tile([C, N], BF16)
        nc.scalar.activation(xb[:, 0:N:XBCH], xth[:, N - (N // XBCH):N],
                             AF.Copy)
        for ci in range(N // XBCH):
            nc.sync.dma_start(xb[:, ci * XBCH:(ci + 1) * XBCH],
                              xb_d.ap()[:, ci * XBCH:(ci + 1) * XBCH])

        # ------- param views into the packed tile -------
        ident = params[:, 0:C]
        wsb = {}
        for wi, nm in enumerate(("Wq1", "Wk1", "Wv1", "Wq2", "Wk2", "Wv2",
                                 "Wq3", "Wk3", "Wv3", "conv0_w")):
            wsb[nm] = params[:, (1 + wi) * C:(2 + wi) * C]
        fc1w = params[:H, 11 * C:12 * C]
        fc2w = params[:, 12 * C:12 * C + H]
        vrows = params[:7, 13 * C:14 * C]

        # ------- constants -------
        dummy = singles.tile([1, 1], F32)
        nc.vector.memset(dummy[:], 1.0)
        nc.scalar.activation(dummy[:], dummy[:], AF.Tanh)  # early table load

        ones_col = singles.tile([C, 1], F32)
        nc.vector.memset(ones_col[:], 1.0)
        ones_row = singles.tile([1, C], F32)
        nc.vector.memset(ones_row[:], 1.0)
        magic = singles.tile([C, K], U32)
        nc.vector.memset(magic[:], 0x5f3759df)

        def rsqrt_newton(v_ap, out_ap, scr_ap, p, w, iters=2):
            ou = out_ap.bitcast(U32)
            nc.vector.tensor_scalar(ou, v_ap.bitcast(U32), 1, None,
                                    ALU.logical_shift_right)
            nc.vector.tensor_tensor(ou, magic[:p, :w], ou, ALU.subtract)
            for _ in range(iters):
                nc.vector.tensor_tensor(scr_ap, out_ap, out_ap, ALU.mult)
                nc.vector.tensor_tensor(scr_ap, scr_ap, v_ap, ALU.mult)
                nc.vector.tensor_scalar(scr_ap, scr_ap, -0.5, 1.5,
                                        ALU.mult, ALU.add)
                nc.vector.tensor_tensor(out_ap, out_ap, scr_ap, ALU.mult)

        # ------- tanh, T-layout first (gates pooled) -------
        wT = singles.tile([C, C], F32)   # [i, q], n = q*128+i
        nc.scalar.activation(wT[:], lgT[:], AF.Tanh)
        wA = singles.tile([C, C], F32)   # [p, f], n = p*128+f
        nc.scalar.activation(wA[:], lg[:], AF.Tanh)

        def build_masks(dst, src, nbins, eng, scrtag):
            for j in range(8):
                lo = -1.0 + 0.25 * j
                eng.tensor_scalar(dst[:, j * C:(j + 1) * C], src[:],
                                  float(lo), None, ALU.is_gt)
            for j in range(7):
                eng.tensor_tensor(dst[:, j * C:(j + 1) * C],
                                  dst[:, j * C:(j + 1) * C],
                                  dst[:, (j + 1) * C:(j + 2) * C],
                                  ALU.subtract)
            neq = scr2.tile([C, C], dst.dtype, tag=scrtag, name=f"neq_{scrtag}")
            eng.tensor_scalar(neq[:], src[:], 0.0, None, ALU.not_equal)
            eng.tensor_tensor(dst[:, 3 * C:4 * C], dst[:, 3 * C:4 * C],
                              neq[:], ALU.mult)
            if nbins > 8:
                eng.memset(dst[:, 8 * C:9 * C], 1.0)

        # T-layout masks on vector (critical: gate pooled matmuls)
        mT = singles.tile([C, 9 * C], BF16)
        build_masks(mT, wT, 9, nc.vector, "neqv")

        # A-layout masks on vector; Mrow roundtrip DMAs on gpsimd rings
        mA = singles.tile([C, 8 * C], BF16)
        build_masks(mA, wA, 8, nc.vector, "neqg")
        mrow_dram = dramp.tile([K, N], BF16)
        for j in range(K):
            nc.gpsimd.dma_start(mrow_dram[j:j + 1, :].rearrange("o n -> (o n)"),
                                mA[:, j * C:(j + 1) * C])
        mrowp = ctx.enter_context(tc.tile_pool(name="mrowp", bufs=3))
        numsA = singles.tile([C, K], F32)
        for j in range(K):
            nc.vector.reduce_sum(numsA[:, j:j + 1], mT[:, j * C:(j + 1) * C],
                                 axis=AX.X)

        # ------- pooled bin sums [C, 9]: lhsT = xth slabs (PE queue head) --
        psB_cm = tc.tile_pool(name="psB", bufs=1, space="PSUM")
        psB = psB_cm.__enter__()
        pooled_ps = psB.tile([C, K + 1], F32)
        for q in range(N // C):
            nc.tensor.matmul(pooled_ps[:], xth[:, q * C:(q + 1) * C],
                             mT[:, q::C],
                             start=(q == 0), stop=(q == N // C - 1))

        # nums as a row + broadcasts (tiny PE ops)
        numsr_ps = psA.tile([1, K], F32, tag="pa_small")
        nc.tensor.matmul(numsr_ps[:], ones_col[:], numsA[:], start=True,
                         stop=True)
        numsr = singles.tile([1, 2 * K], F32)
        nc.vector.tensor_scalar(numsr[:, :K], numsr_ps[:], 1.0, None, ALU.max)
        nc.vector.reciprocal(numsr[:, K:], numsr[:, :K])
        nbc_ps = psA.tile([C, 2 * K], F32, tag="pa_small")
        nc.tensor.matmul(nbc_ps[:], ones_row[:], numsr[:], start=True,
                         stop=True)
        numsbc = singles.tile([C, 2 * K], F32)
        nc.vector.tensor_copy(numsbc[:], nbc_ps[:])

        # ------- weight transposes (PE) + copies (vector) -------
        temp = float(np.sqrt(np.float32(C)))
        wqkT = []
        wvT = []
        for l in range(3):
            qk = singles.tile([C, 2 * C], F32, tag=f"wqkT{l}")
            ps = psA.tile([C, C], F32, tag="pa")
            nc.tensor.transpose(ps[:], wsb[f"Wq{l+1}"][:], ident[:])
            nc.vector.tensor_scalar(qk[:, :C], ps[:], 1.0 / temp, None,
                                    ALU.mult)
            ps = psA.tile([C, C], F32, tag="pa")
            nc.tensor.transpose(ps[:], wsb[f"Wk{l+1}"][:], ident[:])
            nc.vector.tensor_copy(qk[:, C:], ps[:])
            wqkT.append(qk)
            vt = singles.tile([C, C], F32, tag=f"wvT{l}")
            ps = psA.tile([C, C], F32, tag="pa")
            nc.tensor.transpose(ps[:], wsb[f"Wv{l+1}"][:], ident[:])
            nc.vector.tensor_copy(vt[:], ps[:])
            wvT.append(vt)
        convwT = singles.tile([C, C], F32)
        ps = psA.tile([C, C], F32, tag="pa")
        nc.tensor.transpose(ps[:], wsb["conv0_w"][:], ident[:])
        nc.vector.tensor_copy(convwT[:], ps[:])
        convwTb = singles.tile([C, C], BF16)
        nc.vector.tensor_copy(convwTb[:], ps[:])
        fc1wT = singles.tile([C, H], F32)
        ps = psA.tile([C, C], F32, tag="pa")
        nc.tensor.transpose(ps[:, :H], fc1w[:], ident[:H, :H])
        nc.vector.tensor_copy(fc1wT[:], ps[:, :H])
        fc2wT = singles.tile([H, C], F32)
        ps = psA.tile([C, C], F32, tag="pa")
        nc.tensor.transpose(ps[:H, :], fc2w[:], ident[:])
        nc.vector.tensor_copy(fc2wT[:], ps[:H, :])
        ps = psA.tile([C, C], F32, tag="pa")
        nc.tensor.transpose(ps[:, :7], vrows[:], ident[:7, :7])
        vcols = singles.tile([C, 7], F32)
        nc.vector.tensor_copy(vcols[:], ps[:, :7])
        lnw_c, lnb_c = vcols[:, 0:1], vcols[:, 1:2]
        convb_c = vcols[:, 2:3]
        bnw_c, bnb_c = vcols[:, 3:4], vcols[:, 4:5]
        fc1b_c = vcols[:H, 5:6]
        halffc2b_c = singles.tile([C, 1], F32)
        nc.vector.tensor_scalar(halffc2b_c[:], vcols[:, 6:7], 0.5, None,
                                ALU.mult)

        # fea = pooled * (1/nums) columnwise; sumx = pooled col 8
        fea0 = singles.tile([C, K], F32)
        nc.vector.tensor_tensor(fea0[:], pooled_ps[:, :K], numsbc[:, K:],
                                ALU.mult)
        fea = fea0[:]

        # stats partials that only need fea0/pooled
        r123 = singles.tile([C, 3], F32)

        # pre-memset v9 ones columns for all 3 layers
        v9s = []
        for l in range(3):
            v9 = singles.tile([C, K + 1], F32, tag=f"v9{l}")
            nc.vector.memset(v9[:, K:K + 1], 1.0)
            v9s.append(v9)

        # ------- x^2 channel sums, split across engines -------
        nsq = N // SQCH
        xsq_part = singles.tile([C, nsq], F32)
        sqscr = singles.tile([C, SQCH], BF16)
        g = singles.tile([C, N], BF16)
        _sq_next = [0]

        def emit_squares(n, eng="scalar"):
            for _ in range(n):
                ci = _sq_next[0]
                if ci >= nsq:
                    return
                _sq_next[0] += 1
                sl = xb[:, ci * SQCH:(ci + 1) * SQCH]
                if eng == "scalar":
                    nc.scalar.activation(sqscr[:], sl, AF.Square,
                                         accum_out=xsq_part[:, ci:ci + 1])
                else:
                    # g is dead until the R gelus overwrite it; use as scratch
                    nc.vector.scalar_tensor_tensor(
                        g[:, 0:SQCH], sl, 1.0, sl, ALU.mult, ALU.mult,
                        accum_out=xsq_part[:, ci:ci + 1])

        # ------- attention x3 (squares on scalar, xb-gated) -------
        emit_squares(2)
        expT_last = None
        for l in range(3):
            qk_ps = psA.tile([K, 2 * C], F32, tag="pa_small")
            nc.tensor.matmul(qk_ps[:], fea, wqkT[l][:], start=True, stop=True)
            qkT = singles.tile([K, 2 * C], F32, tag=f"qkT{l}")
            nc.vector.tensor_copy(qkT[:], qk_ps[:])
            v_ps = psA.tile([C, K], F32, tag="pa_small")
            nc.tensor.matmul(v_ps[:], wvT[l][:], fea, start=True, stop=True)
            v9 = v9s[l]
            nc.vector.tensor_copy(v9[:, :K], v_ps[:])
            atT_ps = psA.tile([C, C], F32, tag="pa")
            nc.tensor.matmul(atT_ps[:], qkT[:, C:], qkT[:, :C], start=True,
                             stop=True)
            expT = singles.tile([C, C], F32, tag=f"eT{l}")
            nc.scalar.activation(expT[:], atT_ps[:], AF.Exp)
            expT_last = expT
            emit_squares(2)
            ao9_ps = psA.tile([C, K + 1], F32, tag="pa_small")
            nc.tensor.matmul(ao9_ps[:], expT[:], v9[:], start=True, stop=True)
            rse = singles.tile([C, 1], F32, tag=f"rse{l}")
            nc.vector.reciprocal(rse[:], ao9_ps[:, K:K + 1])
            stin = singles.tile([C, 2 * K], F32, tag=f"stin{l}")
            nc.vector.scalar_tensor_tensor(stin[:, :K], ao9_ps[:, :K], rse[:],
                                           fea, ALU.mult, ALU.add)
            nc.vector.tensor_tensor(stin[:, K:], stin[:, :K], stin[:, :K],
                                    ALU.mult)
            st_ps = psA.tile([1, 2 * K], F32, tag="pa_small")
            nc.tensor.matmul(st_ps[:], ones_col[:], stin[:], start=True,
                             stop=True)
            mr = singles.tile([1, 2 * K], F32, tag=f"mr{l}")
            nc.vector.tensor_scalar(mr[:], st_ps[:], 1.0 / C, None, ALU.mult)
            vs8 = singles.tile([1, 2 * K], F32, tag=f"vs8{l}")
            nc.vector.tensor_tensor(vs8[:, K:], mr[:, :K], mr[:, :K], ALU.mult)
            # var + eps in one fused op: (E[x^2] + eps) - mu^2
            nc.vector.scalar_tensor_tensor(vs8[:, :K], mr[:, K:], 1e-6,
                                           vs8[:, K:], ALU.add, ALU.subtract)
            rsqrt_newton(vs8[:, :K], mr[:, K:], vs8[:, K:], 1, K, iters=1)
            bc_ps = psA.tile([C, 2 * K], F32, tag="pa_small")
            nc.tensor.matmul(bc_ps[:], ones_row[:], mr[:], start=True,
                             stop=True)
            fea2 = singles.tile([C, K], F32, tag=f"fea{l+1}")
            nc.vector.tensor_tensor(fea2[:], stin[:, :K], bc_ps[:, :K],
                                    ALU.subtract)
            nc.vector.tensor_tensor(fea2[:], fea2[:], bc_ps[:, K:], ALU.mult)
            nc.vector.tensor_scalar(fea2[:], fea2[:], lnw_c, lnb_c,
                                    ALU.mult, ALU.add)
            if l == 0:
                # stats partials for the *input* levels are not needed; but
                # fill r1/r3 gaps for the final feature later.
                pass
            fea = fea2[:]

        # act-table switch pinned after the last exp via data dependency
        nc.scalar.activation(dummy[:], expT_last[0:1, 0:1], AF.Gelu)
        emit_squares(16)  # any remainder on scalar

        # ------- stats in channel-major orientation -------
        # r1 = sum_j fea*nums, r2 = sum_j fea^2*nums, r3 = sum_j fea*pooled
        feasq = singles.tile([C, K], F32)
        scrK = singles.tile([C, K], F32)
        nc.vector.scalar_tensor_tensor(scrK[:], fea, 1.0, numsbc[:, :K],
                                       ALU.mult, ALU.mult,
                                       accum_out=r123[:, 0:1])
        nc.vector.tensor_tensor(feasq[:], fea, fea, ALU.mult)
        nc.vector.scalar_tensor_tensor(scrK[:], feasq[:], 1.0, numsbc[:, :K],
                                       ALU.mult, ALU.mult,
                                       accum_out=r123[:, 1:2])
        nc.vector.scalar_tensor_tensor(scrK[:], fea, 1.0, pooled_ps[:, :K],
                                       ALU.mult, ALU.mult,
                                       accum_out=r123[:, 2:3])

        # featb [K, C] bf16 for pass R weights
        ftp = psA.tile([C, C], F32, tag="pa")
        nc.tensor.transpose(ftp[:K, :], fea, ident[:])
        featb = singles.tile([K, C], BF16)
        nc.vector.tensor_copy(featb[:], ftp[:K, :])

        # xsq-independent stats first (varA), then the short xsq tail.
        # Fused InstanceNorm*BatchNorm rescale:
        #   rs_i*rs_b = rsqrt(var*(1+1e-5) + 1e-10)   (exact algebra)
        stats = singles.tile([C, 8], F32)
        mu = stats[:, 0:1]
        varA = stats[:, 1:2]
        var2 = stats[:, 2:3]
        s_col = stats[:, 3:4]
        b_col = stats[:, 4:5]
        tmp = stats[:, 5:6]
        rs = stats[:, 6:7]
        var = stats[:, 7:8]
        nc.vector.tensor_tensor(tmp[:], pooled_ps[:, K:K + 1], r123[:, 0:1],
                                ALU.add)
        nc.vector.tensor_scalar(mu[:], tmp[:], 1.0 / N, None, ALU.mult)
        nc.vector.tensor_scalar(tmp[:], r123[:, 2:3], 2.0, None, ALU.mult)
        nc.vector.tensor_tensor(tmp[:], tmp[:], r123[:, 1:2], ALU.add)
        nc.vector.tensor_scalar(tmp[:], tmp[:], 1.0 / N, None, ALU.mult)
        nc.vector.tensor_tensor(varA[:], mu[:], mu[:], ALU.mult)
        nc.vector.tensor_tensor(varA[:], tmp[:], varA[:], ALU.subtract)
        xsq_col = singles.tile([C, 1], F32)
        nc.vector.reduce_sum(xsq_col[:], xsq_part[:], axis=AX.X)
        scr2c = singles.tile([C, 2], F32, tag="nsc")
        nc.vector.tensor_scalar(var[:], xsq_col[:], 1.0 / N, varA[:],
                                ALU.mult, ALU.add)
        nc.vector.tensor_scalar(var2[:], var[:], 1.0 + 1e-5, 1e-10,
                                ALU.mult, ALU.add)
        rsqrt_newton(var2[:], rs[:], scr2c[:, 0:1], C, 1)
        nc.vector.tensor_tensor(s_col[:], rs[:], bnw_c, ALU.mult)
        nc.vector.tensor_tensor(b_col[:], mu[:], s_col[:], ALU.mult)
        nc.vector.tensor_tensor(b_col[:], bnb_c, b_col[:], ALU.subtract)
        # dep1 == 1.0, but data-dependent on the stats tail: threading it
        # through the R adds keeps the in-order vector queue from running
        # all 8 adds ahead of the stats ops (which would stall every gelu).
        dep1 = singles.tile([C, 1], F32)
        nc.vector.tensor_scalar(dep1[:], b_col[:], 0.0, 1.0, ALU.mult,
                                ALU.add)

        psB_cm.__exit__(None, None, None)
        psA_cm.__exit__(None, None, None)

        # ------- pass R: scatter MMs (PE) + x-add (vector) + gelu (scalar) -
        gin = xth  # xth is dead after pooled; reuse its SBUF for x+scatter
        gsum_part = singles.tile([C, N // RT], F32)
        psR_cm = tc.tile_pool(name="psR", bufs=2, space="PSUM")
        psR = psR_cm.__enter__()
        for r in range(N // RT):
            off = r * RT
            mr_t = mrowp.tile([K, RT], BF16, tag="mr", name=f"mr{r}")
            nc.gpsimd.dma_start(mr_t[:], mrow_dram[:, off:off + RT])
            rt_ps = psR.tile([C, RT], F32, tag="pr", name=f"rt_ps{r}")
            for h in range(RT // 512):
                nc.tensor.matmul(rt_ps[:, h * 512:(h + 1) * 512],
                                 featb[:],
                                 mr_t[:, h * 512:(h + 1) * 512],
                                 start=True, stop=True)
            nc.vector.scalar_tensor_tensor(gin[:, off:off + RT], rt_ps[:],
                                           dep1[:], xb[:, off:off + RT],
                                           ALU.mult, ALU.add)
            nc.scalar.activation(g[:, off:off + RT], gin[:, off:off + RT],
                                 AF.Gelu, bias=b_col, scale=s_col,
                                 accum_out=gsum_part[:, r:r + 1])

        # ------- SE gates (psum reused from psR slots) -------
        gps = psR.tile([C, RT], F32, tag="pr", name="gates_ps")
        gsum_col = singles.tile([C, 1], F32)
        nc.vector.reduce_sum(gsum_col[:], gsum_part[:], axis=AX.X)
        nc.tensor.matmul(gps[:, 0:1], convwT[:], gsum_col[:], start=True,
                         stop=True)
        sq = singles.tile([C, 1], F32)
        nc.vector.tensor_scalar(sq[:], gps[:, 0:1], 1.0 / N, convb_c,
                                ALU.mult, ALU.add)
        nc.tensor.matmul(gps[:H, 1:2], fc1wT[:], sq[:], start=True, stop=True)
        f1 = singles.tile([H, 1], F32)
        nc.scalar.activation(f1[:], gps[:H, 1:2], AF.Gelu, bias=fc1b_c)
        nc.tensor.matmul(gps[:, 2:3], fc2wT[:], f1[:], start=True, stop=True)
        f2 = singles.tile([C, 1], F32)
        nc.scalar.activation(f2[:], gps[:, 2:3], AF.Tanh, scale=0.5,
                             bias=halffc2b_c)
        nc.vector.tensor_scalar(f2[:], f2[:], 0.5, 0.5, ALU.mult, ALU.add)
        fb = singles.tile([C, 1], F32)
        nc.vector.tensor_tensor(fb[:], f2[:], convb_c, ALU.mult)
        psR_cm.__exit__(None, None, None)

        # ------- pass F: unscaled conv; f2/fb folded into the output op ----
        with tc.tile_pool(name="psF", bufs=2, space="PSUM") as psF:
            for r in range(N // FT):
                off = r * FT
                cv_ps = psF.tile([C, FT], F32, tag="pf")
                for h in range(FT // 512):
                    nc.tensor.matmul(cv_ps[:, h * 512:(h + 1) * 512],
                                     convwTb[:],
                                     g[:, off + h * 512:off + (h + 1) * 512],
                                     start=True, stop=True)
                ot = och.tile([C, FT], BF16, tag="ot")
                hf = FT // 2
                nc.vector.tensor_scalar(ot[:, :hf], cv_ps[:, :hf], f2[:],
                                        fb[:], ALU.mult, ALU.add)
                nc.scalar.activation(ot[:, hf:], cv_ps[:, hf:], AF.Identity,
                                     bias=fb[:], scale=f2[:])
                nc.sync.dma_start(out_d.ap()[:, off:off + FT], ot[:])


_NC_CACHE = {}


def _get_nc():
    if "nc" not in _NC_CACHE:
        _NC_CACHE["nc"] = build_nc()
    return _NC_CACHE["nc"]


def _prep_inputs(inputs):
    import ml_dtypes
    bf16 = ml_dtypes.bfloat16
    x = np.asarray(inputs["x"], dtype=np.float32)
    logits = np.asarray(inputs["logits"], dtype=np.float32)
    assert x.shape == (B, C, N, 1) and logits.shape == (B, N)
    params = np.zeros((C, 14 * C), dtype=np.float32)
    params[:, 0:C] = np.eye(C, dtype=np.float32)
    for wi, nm in enumerate(("Wq1", "Wk1", "Wv1", "Wq2", "Wk2", "Wv2",
                             "Wq3", "Wk3", "Wv3", "conv0_w")):
        params[:, (1 + wi) * C:(2 + wi) * C] = np.asarray(inputs[nm],
                                                          dtype=np.float32)
    params[:H, 11 * C:12 * C] = np.asarray(inputs["fc1_w"], dtype=np.float32)
    params[:, 12 * C:12 * C + H] = np.asarray(inputs["fc2_w"],
                                              dtype=np.float32)
    for r, nm in enumerate(("ln_w", "ln_b", "conv0_b", "bn_w", "bn_b")):
        params[r, 13 * C:14 * C] = np.asarray(inputs[nm], dtype=np.float32)
    params[5, 13 * C:13 * C + H] = np.asarray(inputs["fc1_b"],
                                              dtype=np.float32)
    params[6, 13 * C:14 * C] = np.asarray(inputs["fc2_b"], dtype=np.float32)
    shared = {"params": params}
    in_maps = []
    for i in range(NCORES):
        m = dict(shared)
        xb = np.ascontiguousarray(x[i, :, :, 0].astype(bf16))
        m["xb"] = xb
        m["xth"] = np.ascontiguousarray(
            xb.reshape(C, N // C, C).transpose(2, 1, 0)).reshape(C, N)
        lgA = logits[i].reshape(C, C)
        m["lg"] = np.ascontiguousarray(lgA)
        m["lgT"] = np.ascontiguousarray(lgA.T)
        in_maps.append(m)
    return in_maps


def kernel(**inputs):
    in_maps = _prep_inputs(inputs)
    nc = _get_nc()
    res = run_bass_kernel_spmd(nc, in_maps, list(range(NCORES))).results
    out = np.stack([res[i]["out"] for i in range(NCORES)], axis=0)
    return out[..., None].astype(np.float32)
